# revision 45
# baseline (speedup 1.0000x reference)
"""Trainium2 Bass kernel for a BFP-quantized ResNet BasicBlock (inference).

Computes, per image (NCHW, C=128, H=W=56):
    out = relu( bn2( conv3x3( q( relu(bn1( conv3x3(q(x), q(w1)) )) ), q(w2)) ) + x )
where q() is HBFP block-floating-point quantization: blocks of 64 contiguous
values (flat row-major) share a power-of-2 scale 2^(floor(log2(max|x|))-7),
mantissas rounded (RNE) to 8 signed bits and clamped to +-127.

Implementation notes:
  * Mantissa rounding via the fp16 magic constant: for t in [-127.44, 127.44],
    fp16(t + 1536) lands in [1408.5, 1663.5) where fp16's ulp is exactly 1, so
    the output conversion itself performs RNE-to-integer. Keeping the whole
    elementwise chain in 16-bit dtypes doubles DVE throughput.
  * The unbias-and-scale step is one fused gpsimd scalar_tensor_tensor:
    u = (t - 1536) * scale_broadcast, with exact results (8-bit mantissa times
    a power of two is exact in bf16).
  * conv3x3 = 9 accumulated matmuls (C_in=128 on partitions) over a zero-
    padded 58-pitch image; the rhs reads only the 56 valid columns per row
    (strided AP), so matmul free size is 448, and PSUM evictions are
    contiguous.
  * bn2 + residual + final relu are folded into conv2: inv2 is folded into
    the quantized w2 (per-output-channel scale), x is DMA-preloaded into each
    PSUM chunk and the matmuls accumulate on top, the eviction applies
    Relu(psum + b2) directly into the output tile.
  * Quantized inputs are written to the padded layout by a Scalar-engine
    copy (one instruction per piece) instead of a 56-rows-per-partition DMA.

Sharding: data-parallel over batch N=64 -> 8 images per NeuronCore, weights
and BN constants replicated. All 8 cores run the same NEFF (SPMD).
"""

import os

os.environ.setdefault("MYCRO_LOCAL_CACHE", "1")

from contextlib import ExitStack
from functools import lru_cache

import numpy as np

import concourse.bass as bass
import concourse.tile as tile
from concourse import bacc, mybir
from concourse.bass_utils import run_bass_kernel_spmd
from concourse.masks import make_identity

P = 128
H = W = 56
HWF = H * W            # 3136 flat pixels per channel
NBX = HWF // 64        # 49 BFP blocks per channel image
WLEN = 128 * 9         # 1152 flat weight row per output channel
NBW = WLEN // 64       # 18 BFP blocks per weight row
PITCH = W + 2          # 58 padded row pitch
PADLEN = PITCH * PITCH + 2  # 3366: [1 pre-pad][58x58 padded image][1 post-pad]
NCHUNK = 7             # 8-row output chunks per image
CHF = 8 * PITCH        # 464 matmul free dim per chunk (contiguous)
CHW = 8 * W            # 448 output elements per chunk
CROUND = 12582912.0    # 1.5 * 2**23  (f32 RNE-to-integer magic constant)
EXPMASK = 0x7F800000
BIAS7 = 7 << 23
C254 = 254 << 23
EGUARD = 50 << 23      # exponent field of 1e-23 (reference's zero-guard)
BN_EPS = 1e-5

F32 = mybir.dt.float32
BF16 = mybir.dt.bfloat16
FP16 = mybir.dt.float16
I32 = mybir.dt.int32
ALU = mybir.AluOpType
ACTF = mybir.ActivationFunctionType
AX = mybir.AxisListType

N_CORES = 8
NIMG = 8  # images per core
NPAD = 3  # padded-image buffers per conv


def _emit_quant(nc, small, t16_pool, src_ap, u3, nb, pieces, lane,
                piece_done=None):
    """BFP-quantize src_ap (f32 [P, nb, 64]) into u3 (bf16 [P, nb, 64]).

    Bit-exact mantissa rounding via the f32 magic constant:
      t = x * rsc               (rsc = 1/s is a power of two -> exact in f32)
      t = (t + 1.5*2^23) - 1.5*2^23           (one dual-op tensor_scalar; the
                                 f32 magic add performs RNE to integer)
      t = clip(t, -127, 127)
      u = t * s                 (8-bit mantissa times power of two: exact
                                 in bf16)

    pieces: list of (block_start, block_count) spans, emitted independently so
    downstream consumers can start before the whole tensor is produced.

    lane: 'dve' runs the whole chain on VectorE; 'mixed' offloads the two
    broadcast multiplies to GpSimd (round/clip stay on VectorE — Q7
    tensor_scalar is far below DVE speed). The reduce and the per-block
    scale-bit ops are always on VectorE.
    """
    t = t16_pool.tile([P, nb * 64], F32, tag=f"t_{nb}")
    tf3 = t[:].rearrange("p (b e) -> p b e", e=64)
    t16 = t16_pool.tile([P, nb * 64], FP16, tag=f"t16_{nb}")
    t3 = t16[:].rearrange("p (b e) -> p b e", e=64)
    bm32 = small.tile([P, nb], F32, tag=f"bm32_{nb}")
    sb = small.tile([P, nb], I32, tag=f"sb{nb}")
    rb = small.tile([P, nb], I32, tag=f"rb{nb}")
    eng = nc.vector if lane == "dve" else nc.gpsimd
    for b0, bn in pieces:
        sl = slice(b0, b0 + bn)
        nc.vector.tensor_reduce(
            out=bm32[:, sl], in_=src_ap[:, sl], axis=AX.X,
            op=ALU.max, apply_absolute_value=True,
        )
        # scale bits = max(exponent field, expfield(1e-23)) - (7 << 23)
        nc.vector.tensor_scalar(sb[:, sl], bm32[:, sl].bitcast(I32), EXPMASK,
                                None, ALU.bitwise_and)
        nc.vector.tensor_scalar(sb[:, sl], sb[:, sl], EGUARD, BIAS7,
                                ALU.max, ALU.subtract)
        # rscale bits = (254 << 23) - scale_bits  -> rscale = 2^(7-e)
        nc.vector.tensor_scalar(rb[:, sl], sb[:, sl], C254, -1,
                                ALU.subtract, ALU.mult)
        rsc = rb[:].bitcast(F32)[:, sl, None].to_broadcast((P, bn, 64))
        eng.tensor_tensor(tf3[:, sl], src_ap[:, sl], rsc, ALU.mult)
        nc.vector.tensor_scalar(t3[:, sl], tf3[:, sl], CROUND, CROUND,
                                ALU.add, ALU.subtract)
        nc.vector.tensor_scalar(t3[:, sl], t3[:, sl], 127.0, -127.0,
                                ALU.min, ALU.max)
        scb = sb[:].bitcast(F32)[:, sl, None].to_broadcast((P, bn, 64))
        eng.tensor_tensor(u3[:, sl], t3[:, sl], scb, ALU.mult)
        if piece_done is not None:
            piece_done(b0, bn)


def _interior(pad_tile):
    """[P, 56, 56] strided view (pitch 58) of the padded tile's interior."""
    base = 1 + PITCH + 1  # (h=0, w=0) -> index 1 + (h+1)*58 + (w+1)
    v = pad_tile[:, base : base + H * PITCH]
    return v.rearrange("p (h w) -> p h w", w=PITCH)[:, :, :W]


def _psv(ps):
    """[P, 8, 56] valid-column view of a [P, 464] PSUM chunk."""
    return ps[:].rearrange("p (r w) -> p r w", w=PITCH)[:, :, 1 : 1 + W]


def _emit_conv(nc, psum_pool, wk, src_pad, evict, preload=None):
    """3x3 conv via 9 accumulated matmuls per 8-row chunk; evict(c, psum).

    preload(c, ps): optionally pre-fill PSUM (residual); matmuls accumulate.
    """
    for c in range(NCHUNK):
        ps = psum_pool.tile([P, CHF], F32, tag="ps")
        if preload is not None:
            preload(c, ps)
        for k in range(9):
            kh, kw = divmod(k, 3)
            s = (c * 8 + kh) * PITCH + kw
            nc.tensor.matmul(
                ps[:], wk[k][:], src_pad[:, s : s + CHF],
                start=(k == 0 and preload is None), stop=(k == 8),
            )
        evict(c, ps)


def build_nc(nimg=NIMG):
    nc = bacc.Bacc("TRN2", target_bir_lowering=False, debug=False,
                   enable_asserts=False)

    x_d = nc.dram_tensor("x", [nimg, P, H, W], F32, kind="ExternalInput").ap()
    w1_d = nc.dram_tensor("w1", [P, P, 3, 3], F32, kind="ExternalInput").ap()
    w2_d = nc.dram_tensor("w2", [P, P, 3, 3], F32, kind="ExternalInput").ap()
    bn_d = {
        name: nc.dram_tensor(name, [P], F32, kind="ExternalInput").ap()
        for name in ("gamma1", "beta1", "mean1", "var1",
                     "gamma2", "beta2", "mean2", "var2")
    }
    out_d = nc.dram_tensor("out", [nimg, P, H, W], F32, kind="ExternalOutput").ap()

    with tile.TileContext(nc) as tc, ExitStack() as ctx:
        const = ctx.enter_context(tc.tile_pool(name="const", bufs=1))
        small = ctx.enter_context(tc.tile_pool(name="small", bufs=6))
        t16_p = ctx.enter_context(tc.tile_pool(name="t16", bufs=2))
        pads = ctx.enter_context(tc.tile_pool(name="pads", bufs=1))
        xraw_p = ctx.enter_context(tc.tile_pool(name="xraw", bufs=3))

        xraws = [None] * nimg

        def load(n):
            xraw = xraw_p.tile([P, HWF], F32, tag="xraw", name=f"xraw{n}")
            xraws[n] = xraw
            nc.sync.dma_start(xraw[:], x_d[n].rearrange("c h w -> c (h w)"))

        # first images stream in while weights are being prepared
        load(0)
        load(1)

        # ---- setup: BN constants, weights quant (+bn2 fold) + transpose ----
        with tc.tile_pool(name="setup", bufs=1) as setup, \
             tc.tile_pool(name="psum_setup", bufs=2, space="PSUM") as psum_setup:
            ident = const.tile([P, P], BF16, tag="ident")
            make_identity(nc, ident[:])
            eps_b = const.tile([P, 1], F32, tag="eps_b")
            nc.vector.memset(eps_b[:], BN_EPS)

            bnc = {}
            for name in ("gamma1", "beta1", "mean1", "var1",
                         "gamma2", "beta2", "mean2", "var2"):
                t = setup.tile([P, 1], F32, tag=f"bn_{name}")
                nc.sync.dma_start(t[:], bn_d[name][:, None])
                bnc[name] = t
            invb = []
            for i in ("1", "2"):
                s = setup.tile([P, 1], F32, tag=f"sd{i}")
                nc.scalar.activation(s[:], bnc[f"var{i}"][:], ACTF.Sqrt, bias=eps_b[:])
                r = setup.tile([P, 1], F32, tag=f"rs{i}")
                nc.vector.reciprocal(r[:], s[:])
                inv = const.tile([P, 1], F32, tag=f"inv{i}")
                nc.vector.tensor_tensor(inv[:], bnc[f"gamma{i}"][:], r[:], ALU.mult)
                mi = setup.tile([P, 1], F32, tag=f"mi{i}")
                nc.vector.tensor_tensor(mi[:], bnc[f"mean{i}"][:], inv[:], ALU.mult)
                b = const.tile([P, 1], F32, tag=f"b{i}")
                nc.vector.tensor_tensor(b[:], bnc[f"beta{i}"][:], mi[:], ALU.subtract)
                invb.append((inv, b))
            (inv1, b1), (inv2, b2) = invb

            wks = []
            for wi, w_d in enumerate((w1_d, w2_d)):
                wraw = setup.tile([P, WLEN], F32, tag=f"wraw{wi}")
                nc.sync.dma_start(wraw[:], w_d.rearrange("o i kh kw -> o (i kh kw)"))
                wq = setup.tile([P, WLEN], BF16, tag=f"wq{wi}")
                _emit_quant(nc, small, setup,
                            wraw[:].rearrange("p (b e) -> p b e", e=64),
                            wq[:].rearrange("p (b e) -> p b e", e=64),
                            NBW, [(0, NBW)], lane="mixed")
                if wi == 1:
                    # fold bn2's per-output-channel scale into w2
                    nc.scalar.activation(wq[:], wq[:], ACTF.Identity,
                                         scale=inv2[:])
                # per-offset lhsT tiles: w[k][i, o] = wq[o, i*9+k]
                wq_v = wq[:].rearrange("p (i k) -> p k i", k=9)
                wk = []
                for k in range(9):
                    pt = psum_setup.tile([P, P], BF16, tag="tps")
                    nc.tensor.transpose(pt[:], wq_v[:, k, :], ident[:])
                    wt = const.tile([P, P], BF16, tag=f"w{wi}k{k}")
                    nc.scalar.copy(wt[:], pt[:])
                    wk.append(wt)
                wks.append(wk)
            w1k, w2k = wks

        xq_pads = [pads.tile([P, PADLEN], BF16, tag=f"xqp{i}", name=f"xqp{i}")
                   for i in range(NPAD)]
        mq_pads = [pads.tile([P, PADLEN], BF16, tag=f"mqp{i}", name=f"mqp{i}")
                   for i in range(NPAD)]
        for t in (*xq_pads, *mq_pads):
            nc.scalar.memzero(t[:])

        u_p = ctx.enter_context(tc.tile_pool(name="u", bufs=2))
        mid_p = ctx.enter_context(tc.tile_pool(name="mid", bufs=2))
        out_p = ctx.enter_context(tc.tile_pool(name="outp", bufs=2))
        psum1_p = ctx.enter_context(tc.tile_pool(name="psum1", bufs=4, space="PSUM"))
        psum2_p = ctx.enter_context(tc.tile_pool(name="psum2", bufs=4, space="PSUM"))

        mids = [None] * nimg
        outs = [None] * nimg

        def quant1(n, lane="mixed"):
            u = u_p.tile([P, HWF], BF16, tag="u", name=f"u{n}")
            _emit_quant(nc, small, t16_p,
                        xraws[n][:].rearrange("p (b e) -> p b e", e=64),
                        u[:].rearrange("p (b e) -> p b e", e=64),
                        NBX, [(0, NBX)], lane=lane)
            nc.sync.dma_start(_interior(xq_pads[n % NPAD]),
                              u[:].rearrange("p (h w) -> p h w", w=W))

        def conv1(n):
            mid = mid_p.tile([P, HWF], F32, tag="mid", name=f"mid{n}")
            mids[n] = mid

            def evict1(c, ps):
                ov = mid[:, c * CHW : (c + 1) * CHW].rearrange(
                    "p (r w) -> p r w", w=W)
                nc.scalar.activation(ov, _psv(ps),
                                     ACTF.Relu, bias=b1[:], scale=inv1[:])

            _emit_conv(nc, psum1_p, w1k, xq_pads[n % NPAD][:], evict1)

        def quant2(n):
            u2 = u_p.tile([P, HWF], BF16, tag="u2", name=f"u2_{n}")
            # pieces aligned to evict1's chunks (3/6/7): the last piece is a
            # single chunk so the image-boundary critical chain is short
            iv = _interior(mq_pads[n % NPAD])
            u2v = u2[:].rearrange("p (h w) -> p h w", w=W)

            _emit_quant(nc, small, t16_p,
                        mids[n][:].rearrange("p (b e) -> p b e", e=64),
                        u2[:].rearrange("p (b e) -> p b e", e=64),
                        NBX, [(0, 21), (21, 21), (42, 7)], lane="dve")
            nc.sync.dma_start(iv, u2v)

        def conv2(n):
            out_t = out_p.tile([P, HWF], F32, tag="out", name=f"out{n}")
            outs[n] = out_t
            xraw = xraws[n]

            def evict2(c, ps):
                sl = slice(c * CHW, (c + 1) * CHW)
                ov = out_t[:, sl].rearrange("p (r w) -> p r w", w=W)
                # bn2 (inv2 folded into w2) + residual add; relu after
                nc.scalar.activation(ov, _psv(ps), ACTF.Identity, bias=b2[:])
                nc.gpsimd.tensor_tensor(out_t[:, sl], out_t[:, sl],
                                        xraw[:, sl], ALU.add)
                nc.scalar.activation(out_t[:, sl], out_t[:, sl], ACTF.Relu)

            _emit_conv(nc, psum2_p, w2k, mq_pads[n % NPAD][:], evict2)
            nc.sync.dma_start(out_d[n].rearrange("c h w -> c (h w)"), out_t[:])

        # Software-pipelined emission ordered by criticality: the
        # latency-critical quant2(n) -> conv2(n) chain first each iteration
        # (pure DVE lane), conv1(n+1) as PE filler, input load/quant two
        # images ahead (pure GpSimd lane after the DVE reduce head, so the
        # two in-order queues never block each other).
        quant1(0, lane="dve")
        quant1(1, lane="dve")
        conv1(0)
        for n in range(nimg):
            quant2(n)
            if n + 1 < nimg:
                conv1(n + 1)
            conv2(n)
            if n + 2 < nimg:
                load(n + 2)
                quant1(n + 2, lane="mixed")

    nc.compile()
    return nc


@lru_cache(maxsize=1)
def _get_nc():
    return build_nc(NIMG)


def kernel(x, w1, w2, gamma1, beta1, mean1, var1,
           gamma2, beta2, mean2, var2, _trace=False):
    f = lambda a: np.ascontiguousarray(np.asarray(a, dtype=np.float32))
    x = f(x)
    n_total = x.shape[0]
    assert n_total == N_CORES * NIMG, x.shape
    xs = x.reshape(N_CORES, NIMG, P, H, W)
    rep = {
        "w1": f(w1), "w2": f(w2),
        "gamma1": f(gamma1), "beta1": f(beta1), "mean1": f(mean1), "var1": f(var1),
        "gamma2": f(gamma2), "beta2": f(beta2), "mean2": f(mean2), "var2": f(var2),
    }
    in_maps = [{"x": np.ascontiguousarray(xs[c]), **rep} for c in range(N_CORES)]
    nc = _get_nc()
    res = run_bass_kernel_spmd(nc, in_maps, core_ids=list(range(N_CORES)),
                               trace=_trace)
    out = np.concatenate([res.results[c]["out"] for c in range(N_CORES)], axis=0)
    if _trace:
        kernel.last_result = res
    return out.reshape(n_total, P, H, W)


# revision 47
# speedup vs baseline: 1.0348x; 1.0348x over previous
"""Trainium2 Bass kernel for a BFP-quantized ResNet BasicBlock (inference).

Computes, per image (NCHW, C=128, H=W=56):
    out = relu( bn2( conv3x3( q( relu(bn1( conv3x3(q(x), q(w1)) )) ), q(w2)) ) + x )
where q() is HBFP block-floating-point quantization: blocks of 64 contiguous
values (flat row-major) share a power-of-2 scale 2^(floor(log2(max|x|))-7),
mantissas rounded (RNE) to 8 signed bits and clamped to +-127.

Implementation notes:
  * Mantissa rounding via the fp16 magic constant: for t in [-127.44, 127.44],
    fp16(t + 1536) lands in [1408.5, 1663.5) where fp16's ulp is exactly 1, so
    the output conversion itself performs RNE-to-integer. Keeping the whole
    elementwise chain in 16-bit dtypes doubles DVE throughput.
  * The unbias-and-scale step is one fused gpsimd scalar_tensor_tensor:
    u = (t - 1536) * scale_broadcast, with exact results (8-bit mantissa times
    a power of two is exact in bf16).
  * conv3x3 = 9 accumulated matmuls (C_in=128 on partitions) over a zero-
    padded 58-pitch image; the rhs reads only the 56 valid columns per row
    (strided AP), so matmul free size is 448, and PSUM evictions are
    contiguous.
  * bn2 + residual + final relu are folded into conv2: inv2 is folded into
    the quantized w2 (per-output-channel scale), x is DMA-preloaded into each
    PSUM chunk and the matmuls accumulate on top, the eviction applies
    Relu(psum + b2) directly into the output tile.
  * Quantized inputs are written to the padded layout by a Scalar-engine
    copy (one instruction per piece) instead of a 56-rows-per-partition DMA.

Sharding: data-parallel over batch N=64 -> 8 images per NeuronCore, weights
and BN constants replicated. All 8 cores run the same NEFF (SPMD).
"""

import os

os.environ.setdefault("MYCRO_LOCAL_CACHE", "1")

from contextlib import ExitStack
from functools import lru_cache

import numpy as np

import concourse.bass as bass
import concourse.tile as tile
from concourse import bacc, mybir
from concourse.bass_utils import run_bass_kernel_spmd
from concourse.masks import make_identity

P = 128
H = W = 56
HWF = H * W            # 3136 flat pixels per channel
NBX = HWF // 64        # 49 BFP blocks per channel image
WLEN = 128 * 9         # 1152 flat weight row per output channel
NBW = WLEN // 64       # 18 BFP blocks per weight row
PITCH = W + 2          # 58 padded row pitch
PADLEN = PITCH * PITCH + 2  # 3366: [1 pre-pad][58x58 padded image][1 post-pad]
NCHUNK = 7             # 8-row output chunks per image
CHF = 8 * PITCH        # 464 matmul free dim per chunk (contiguous)
CHW = 8 * W            # 448 output elements per chunk
CROUND = 12582912.0    # 1.5 * 2**23  (f32 RNE-to-integer magic constant)
EXPMASK = 0x7F800000
BIAS7 = 7 << 23
C254 = 254 << 23
EGUARD = 50 << 23      # exponent field of 1e-23 (reference's zero-guard)
BN_EPS = 1e-5

F32 = mybir.dt.float32
BF16 = mybir.dt.bfloat16
FP16 = mybir.dt.float16
I32 = mybir.dt.int32
ALU = mybir.AluOpType
ACTF = mybir.ActivationFunctionType
AX = mybir.AxisListType

N_CORES = 8
NIMG = 8  # images per core
NPAD = 3  # padded-image buffers per conv


def _emit_quant(nc, small, t16_pool, src_ap, u3, nb, pieces, lane,
                piece_done=None):
    """BFP-quantize src_ap (f32 [P, nb, 64]) into u3 (bf16 [P, nb, 64]).

    Bit-exact mantissa rounding via the f32 magic constant:
      t = x * rsc               (rsc = 1/s is a power of two -> exact in f32)
      t = (t + 1.5*2^23) - 1.5*2^23           (one dual-op tensor_scalar; the
                                 f32 magic add performs RNE to integer)
      t = clip(t, -127, 127)
      u = t * s                 (8-bit mantissa times power of two: exact
                                 in bf16)

    pieces: list of (block_start, block_count) spans, emitted independently so
    downstream consumers can start before the whole tensor is produced.

    lane: 'dve' runs the whole chain on VectorE; 'mixed' offloads the two
    broadcast multiplies to GpSimd (round/clip stay on VectorE — Q7
    tensor_scalar is far below DVE speed). The reduce and the per-block
    scale-bit ops are always on VectorE.
    """
    t = t16_pool.tile([P, nb * 64], F32, tag=f"t_{nb}")
    tf3 = t[:].rearrange("p (b e) -> p b e", e=64)
    t16 = t16_pool.tile([P, nb * 64], FP16, tag=f"t16_{nb}")
    t3 = t16[:].rearrange("p (b e) -> p b e", e=64)
    bm32 = small.tile([P, nb], F32, tag=f"bm32_{nb}")
    sb = small.tile([P, nb], I32, tag=f"sb{nb}")
    rb = small.tile([P, nb], I32, tag=f"rb{nb}")
    eng = nc.vector if lane == "dve" else nc.gpsimd
    for b0, bn in pieces:
        sl = slice(b0, b0 + bn)
        nc.vector.tensor_reduce(
            out=bm32[:, sl], in_=src_ap[:, sl], axis=AX.X,
            op=ALU.max, apply_absolute_value=True,
        )
        # scale bits = max(exponent field, expfield(1e-23)) - (7 << 23)
        nc.vector.tensor_scalar(sb[:, sl], bm32[:, sl].bitcast(I32), EXPMASK,
                                None, ALU.bitwise_and)
        nc.vector.tensor_scalar(sb[:, sl], sb[:, sl], EGUARD, BIAS7,
                                ALU.max, ALU.subtract)
        # rscale bits = (254 << 23) - scale_bits  -> rscale = 2^(7-e)
        nc.vector.tensor_scalar(rb[:, sl], sb[:, sl], C254, -1,
                                ALU.subtract, ALU.mult)
        rsc = rb[:].bitcast(F32)[:, sl, None].to_broadcast((P, bn, 64))
        eng.tensor_tensor(tf3[:, sl], src_ap[:, sl], rsc, ALU.mult)
        nc.vector.tensor_scalar(t3[:, sl], tf3[:, sl], CROUND, CROUND,
                                ALU.add, ALU.subtract)
        nc.vector.tensor_scalar(t3[:, sl], t3[:, sl], 127.0, -127.0,
                                ALU.min, ALU.max)
        scb = sb[:].bitcast(F32)[:, sl, None].to_broadcast((P, bn, 64))
        eng.tensor_tensor(u3[:, sl], t3[:, sl], scb, ALU.mult)
        if piece_done is not None:
            piece_done(b0, bn)


def _interior(pad_tile):
    """[P, 56, 56] strided view (pitch 58) of the padded tile's interior."""
    base = 1 + PITCH + 1  # (h=0, w=0) -> index 1 + (h+1)*58 + (w+1)
    v = pad_tile[:, base : base + H * PITCH]
    return v.rearrange("p (h w) -> p h w", w=PITCH)[:, :, :W]


def _psv(ps):
    """[P, 8, 56] valid-column view of a [P, 464] PSUM chunk."""
    return ps[:].rearrange("p (r w) -> p r w", w=PITCH)[:, :, 1 : 1 + W]


def _emit_conv(nc, psum_pool, wk, src_pad, evict, preload=None):
    """3x3 conv via 9 accumulated matmuls per 8-row chunk; evict(c, psum).

    preload(c, ps): optionally pre-fill PSUM (residual); matmuls accumulate.
    """
    for c in range(NCHUNK):
        ps = psum_pool.tile([P, CHF], F32, tag="ps")
        if preload is not None:
            preload(c, ps)
        for k in range(9):
            kh, kw = divmod(k, 3)
            s = (c * 8 + kh) * PITCH + kw
            nc.tensor.matmul(
                ps[:], wk[k][:], src_pad[:, s : s + CHF],
                start=(k == 0 and preload is None), stop=(k == 8),
            )
        evict(c, ps)


def build_nc(nimg=NIMG):
    nc = bacc.Bacc("TRN2", target_bir_lowering=False, debug=False,
                   enable_asserts=False)

    x_d = nc.dram_tensor("x", [nimg, P, H, W], F32, kind="ExternalInput").ap()
    w1_d = nc.dram_tensor("w1", [P, P, 3, 3], F32, kind="ExternalInput").ap()
    w2_d = nc.dram_tensor("w2", [P, P, 3, 3], F32, kind="ExternalInput").ap()
    bn_d = {
        name: nc.dram_tensor(name, [P], F32, kind="ExternalInput").ap()
        for name in ("gamma1", "beta1", "mean1", "var1",
                     "gamma2", "beta2", "mean2", "var2")
    }
    out_d = nc.dram_tensor("out", [nimg, P, H, W], F32, kind="ExternalOutput").ap()

    with tile.TileContext(nc) as tc, ExitStack() as ctx:
        const = ctx.enter_context(tc.tile_pool(name="const", bufs=1))
        small = ctx.enter_context(tc.tile_pool(name="small", bufs=6))
        t16_p = ctx.enter_context(tc.tile_pool(name="t16", bufs=2))
        pads = ctx.enter_context(tc.tile_pool(name="pads", bufs=1))
        xraw_p = ctx.enter_context(tc.tile_pool(name="xraw", bufs=3))

        xraws = [None] * nimg

        def load(n):
            xraw = xraw_p.tile([P, HWF], F32, tag="xraw", name=f"xraw{n}")
            xraws[n] = xraw
            nc.sync.dma_start(xraw[:], x_d[n].rearrange("c h w -> c (h w)"))

        # first images stream in while weights are being prepared
        load(0)
        load(1)

        # ---- setup: BN constants, weights quant (+bn2 fold) + transpose ----
        with tc.tile_pool(name="setup", bufs=1) as setup, \
             tc.tile_pool(name="psum_setup", bufs=2, space="PSUM") as psum_setup:
            ident = const.tile([P, P], BF16, tag="ident")
            make_identity(nc, ident[:])
            eps_b = const.tile([P, 1], F32, tag="eps_b")
            nc.vector.memset(eps_b[:], BN_EPS)

            bnc = {}
            for name in ("gamma1", "beta1", "mean1", "var1",
                         "gamma2", "beta2", "mean2", "var2"):
                t = setup.tile([P, 1], F32, tag=f"bn_{name}")
                nc.sync.dma_start(t[:], bn_d[name][:, None])
                bnc[name] = t
            invb = []
            for i in ("1", "2"):
                s = setup.tile([P, 1], F32, tag=f"sd{i}")
                nc.scalar.activation(s[:], bnc[f"var{i}"][:], ACTF.Sqrt, bias=eps_b[:])
                r = setup.tile([P, 1], F32, tag=f"rs{i}")
                nc.vector.reciprocal(r[:], s[:])
                inv = const.tile([P, 1], F32, tag=f"inv{i}")
                nc.vector.tensor_tensor(inv[:], bnc[f"gamma{i}"][:], r[:], ALU.mult)
                mi = setup.tile([P, 1], F32, tag=f"mi{i}")
                nc.vector.tensor_tensor(mi[:], bnc[f"mean{i}"][:], inv[:], ALU.mult)
                b = const.tile([P, 1], F32, tag=f"b{i}")
                nc.vector.tensor_tensor(b[:], bnc[f"beta{i}"][:], mi[:], ALU.subtract)
                invb.append((inv, b))
            (inv1, b1), (inv2, b2) = invb

            wks = []
            for wi, w_d in enumerate((w1_d, w2_d)):
                wraw = setup.tile([P, WLEN], F32, tag=f"wraw{wi}")
                nc.sync.dma_start(wraw[:], w_d.rearrange("o i kh kw -> o (i kh kw)"))
                wq = setup.tile([P, WLEN], BF16, tag=f"wq{wi}")
                _emit_quant(nc, small, setup,
                            wraw[:].rearrange("p (b e) -> p b e", e=64),
                            wq[:].rearrange("p (b e) -> p b e", e=64),
                            NBW, [(0, NBW)], lane="mixed")
                if wi == 1:
                    # fold bn2's per-output-channel scale into w2
                    nc.scalar.activation(wq[:], wq[:], ACTF.Identity,
                                         scale=inv2[:])
                # per-offset lhsT tiles: w[k][i, o] = wq[o, i*9+k]
                wq_v = wq[:].rearrange("p (i k) -> p k i", k=9)
                wk = []
                for k in range(9):
                    pt = psum_setup.tile([P, P], BF16, tag="tps")
                    nc.tensor.transpose(pt[:], wq_v[:, k, :], ident[:])
                    wt = const.tile([P, P], BF16, tag=f"w{wi}k{k}")
                    nc.scalar.copy(wt[:], pt[:])
                    wk.append(wt)
                wks.append(wk)
            w1k, w2k = wks

        xq_pads = [pads.tile([P, PADLEN], BF16, tag=f"xqp{i}", name=f"xqp{i}")
                   for i in range(NPAD)]
        mq_pads = [pads.tile([P, PADLEN], BF16, tag=f"mqp{i}", name=f"mqp{i}")
                   for i in range(NPAD)]
        for t in (*xq_pads, *mq_pads):
            nc.scalar.memzero(t[:])

        u_p = ctx.enter_context(tc.tile_pool(name="u", bufs=2))
        mid_p = ctx.enter_context(tc.tile_pool(name="mid", bufs=2))
        out_p = ctx.enter_context(tc.tile_pool(name="outp", bufs=2))
        psum1_p = ctx.enter_context(tc.tile_pool(name="psum1", bufs=4, space="PSUM"))
        psum2_p = ctx.enter_context(tc.tile_pool(name="psum2", bufs=4, space="PSUM"))

        mids = [None] * nimg
        outs = [None] * nimg

        def quant1(n, lane="mixed"):
            u = u_p.tile([P, HWF], BF16, tag="u", name=f"u{n}")
            _emit_quant(nc, small, t16_p,
                        xraws[n][:].rearrange("p (b e) -> p b e", e=64),
                        u[:].rearrange("p (b e) -> p b e", e=64),
                        NBX, [(0, NBX)], lane=lane)
            nc.sync.dma_start(_interior(xq_pads[n % NPAD]),
                              u[:].rearrange("p (h w) -> p h w", w=W))

        def conv1(n):
            mid = mid_p.tile([P, HWF], F32, tag="mid", name=f"mid{n}")
            mids[n] = mid

            def evict1(c, ps):
                ov = mid[:, c * CHW : (c + 1) * CHW].rearrange(
                    "p (r w) -> p r w", w=W)
                nc.scalar.activation(ov, _psv(ps),
                                     ACTF.Relu, bias=b1[:], scale=inv1[:])

            _emit_conv(nc, psum1_p, w1k, xq_pads[n % NPAD][:], evict1)

        def quant2(n):
            u2 = u_p.tile([P, HWF], BF16, tag="u2", name=f"u2_{n}")
            # pieces aligned to evict1's chunks (3/6/7): the last piece is a
            # single chunk so the image-boundary critical chain is short
            iv = _interior(mq_pads[n % NPAD])
            u2v = u2[:].rearrange("p (h w) -> p h w", w=W)

            def reloc(b0, bn):
                r0, r1 = b0 * 64 // W, (b0 + bn) * 64 // W
                nc.sync.dma_start(iv[:, r0:r1, :], u2v[:, r0:r1, :])

            _emit_quant(nc, small, t16_p,
                        mids[n][:].rearrange("p (b e) -> p b e", e=64),
                        u2[:].rearrange("p (b e) -> p b e", e=64),
                        NBX, [(0, 21), (21, 21), (42, 7)], lane="dve",
                        piece_done=reloc)

        def conv2(n):
            out_t = out_p.tile([P, HWF], F32, tag="out", name=f"out{n}")
            outs[n] = out_t
            xraw = xraws[n]

            def evict2(c, ps):
                sl = slice(c * CHW, (c + 1) * CHW)
                ov = out_t[:, sl].rearrange("p (r w) -> p r w", w=W)
                # bn2 (inv2 folded into w2); residual add + relu come after
                nc.scalar.activation(ov, _psv(ps), ACTF.Identity, bias=b2[:])

            _emit_conv(nc, psum2_p, w2k, mq_pads[n % NPAD][:], evict2)
            # out = relu(bn2(conv2) + x), halves so the DMA can start early
            for sl in (slice(0, 4 * CHW), slice(4 * CHW, HWF)):
                nc.gpsimd.tensor_tensor(out_t[:, sl], out_t[:, sl],
                                        xraw[:, sl], ALU.add)
                nc.scalar.activation(out_t[:, sl], out_t[:, sl], ACTF.Relu)
                nc.sync.dma_start(
                    out_d[n].rearrange("c h w -> c (h w)")[:, sl],
                    out_t[:, sl])

        # Software-pipelined emission ordered by criticality: the
        # latency-critical quant2(n) -> conv2(n) chain first each iteration
        # (pure DVE lane), conv1(n+1) as PE filler, input load/quant two
        # images ahead (pure GpSimd lane after the DVE reduce head, so the
        # two in-order queues never block each other).
        quant1(0, lane="dve")
        quant1(1, lane="dve")
        conv1(0)
        for n in range(nimg):
            quant2(n)
            if n + 1 < nimg:
                conv1(n + 1)
            conv2(n)
            if n + 2 < nimg:
                load(n + 2)
                quant1(n + 2, lane="mixed")

    nc.compile()
    return nc


@lru_cache(maxsize=1)
def _get_nc():
    return build_nc(NIMG)


def kernel(x, w1, w2, gamma1, beta1, mean1, var1,
           gamma2, beta2, mean2, var2, _trace=False):
    f = lambda a: np.ascontiguousarray(np.asarray(a, dtype=np.float32))
    x = f(x)
    n_total = x.shape[0]
    assert n_total == N_CORES * NIMG, x.shape
    xs = x.reshape(N_CORES, NIMG, P, H, W)
    rep = {
        "w1": f(w1), "w2": f(w2),
        "gamma1": f(gamma1), "beta1": f(beta1), "mean1": f(mean1), "var1": f(var1),
        "gamma2": f(gamma2), "beta2": f(beta2), "mean2": f(mean2), "var2": f(var2),
    }
    in_maps = [{"x": np.ascontiguousarray(xs[c]), **rep} for c in range(N_CORES)]
    nc = _get_nc()
    res = run_bass_kernel_spmd(nc, in_maps, core_ids=list(range(N_CORES)),
                               trace=_trace)
    out = np.concatenate([res.results[c]["out"] for c in range(N_CORES)], axis=0)
    if _trace:
        kernel.last_result = res
    return out.reshape(n_total, P, H, W)


# revision 53
# speedup vs baseline: 1.4178x; 1.3701x over previous
"""Trainium2 Bass kernel for a BFP-quantized ResNet BasicBlock (inference).

Computes, per image (NCHW, C=128, H=W=56):
    out = relu( bn2( conv3x3( q( relu(bn1( conv3x3(q(x), q(w1)) )) ), q(w2)) ) + x )
where q() is HBFP block-floating-point quantization: blocks of 64 contiguous
values (flat row-major) share a power-of-2 scale 2^(floor(log2(max|x|))-7),
mantissas rounded (RNE) to 8 signed bits and clamped to +-127.

Implementation notes:
  * Mantissa rounding via the fp16 magic constant: for t in [-127.44, 127.44],
    fp16(t + 1536) lands in [1408.5, 1663.5) where fp16's ulp is exactly 1, so
    the output conversion itself performs RNE-to-integer. Keeping the whole
    elementwise chain in 16-bit dtypes doubles DVE throughput.
  * The unbias-and-scale step is one fused gpsimd scalar_tensor_tensor:
    u = (t - 1536) * scale_broadcast, with exact results (8-bit mantissa times
    a power of two is exact in bf16).
  * conv3x3 = 9 accumulated matmuls (C_in=128 on partitions) over a zero-
    padded 58-pitch image; the rhs reads only the 56 valid columns per row
    (strided AP), so matmul free size is 448, and PSUM evictions are
    contiguous.
  * bn2 + residual + final relu are folded into conv2: inv2 is folded into
    the quantized w2 (per-output-channel scale), x is DMA-preloaded into each
    PSUM chunk and the matmuls accumulate on top, the eviction applies
    Relu(psum + b2) directly into the output tile.
  * Quantized inputs are written to the padded layout by a Scalar-engine
    copy (one instruction per piece) instead of a 56-rows-per-partition DMA.

Sharding: data-parallel over batch N=64 -> 8 images per NeuronCore, weights
and BN constants replicated. All 8 cores run the same NEFF (SPMD).
"""

import os

os.environ.setdefault("MYCRO_LOCAL_CACHE", "1")

from contextlib import ExitStack
from functools import lru_cache

import numpy as np

import concourse.bass as bass
import concourse.tile as tile
from concourse import bacc, mybir
from concourse.bass_utils import run_bass_kernel_spmd
from concourse.masks import make_identity

P = 128
H = W = 56
HWF = H * W            # 3136 flat pixels per channel
NBX = HWF // 64        # 49 BFP blocks per channel image
WLEN = 128 * 9         # 1152 flat weight row per output channel
NBW = WLEN // 64       # 18 BFP blocks per weight row
PITCH = W + 2          # 58 padded row pitch
PADLEN = PITCH * PITCH + 2  # 3366: [1 pre-pad][58x58 padded image][1 post-pad]
NCHUNK = 7             # 8-row output chunks per image
CHF = 8 * PITCH        # 464 matmul free dim per chunk (contiguous)
CHW = 8 * W            # 448 output elements per chunk
CROUND = 12582912.0    # 1.5 * 2**23  (f32 RNE-to-integer magic constant)
EXPMASK = 0x7F800000
BIAS7 = 7 << 23
C254 = 254 << 23
EGUARD = 50 << 23      # exponent field of 1e-23 (reference's zero-guard)
BN_EPS = 1e-5

F32 = mybir.dt.float32
BF16 = mybir.dt.bfloat16
FP16 = mybir.dt.float16
I32 = mybir.dt.int32
ALU = mybir.AluOpType
ACTF = mybir.ActivationFunctionType
AX = mybir.AxisListType

N_CORES = 8
NIMG = 8  # images per core
NPAD = 3  # padded-image buffers per conv


def _emit_quant(nc, small, t16_pool, src_ap, u3, nb, pieces, lane,
                piece_done=None):
    """BFP-quantize src_ap (f32 [P, nb, 64]) into u3 (bf16 [P, nb, 64]).

    Bit-exact mantissa rounding via the f32 magic constant:
      t = x * rsc               (rsc = 1/s is a power of two -> exact in f32)
      t = (t + 1.5*2^23) - 1.5*2^23           (one dual-op tensor_scalar; the
                                 f32 magic add performs RNE to integer)
      t = clip(t, -127, 127)
      u = t * s                 (8-bit mantissa times power of two: exact
                                 in bf16)

    pieces: list of (block_start, block_count) spans, emitted independently so
    downstream consumers can start before the whole tensor is produced.

    lane: 'dve' runs the whole chain on VectorE; 'mixed' offloads the two
    broadcast multiplies to GpSimd; 'dve_gs' only the final scale multiply
    (round/clip stay on VectorE — Q7 tensor_scalar is far below DVE speed).
    The reduce and the per-block scale-bit ops are always on VectorE.
    """
    t = t16_pool.tile([P, nb * 64], F32, tag=f"t_{nb}")
    tf3 = t[:].rearrange("p (b e) -> p b e", e=64)
    t16 = t16_pool.tile([P, nb * 64], FP16, tag=f"t16_{nb}")
    t3 = t16[:].rearrange("p (b e) -> p b e", e=64)
    bm32 = small.tile([P, nb], F32, tag=f"bm32_{nb}")
    sb = small.tile([P, nb], I32, tag=f"sb{nb}")
    rb = small.tile([P, nb], I32, tag=f"rb{nb}")
    eng = nc.vector if lane in ("dve", "dve_gs") else nc.gpsimd
    seng = nc.vector if lane == "dve" else nc.gpsimd
    for b0, bn in pieces:
        sl = slice(b0, b0 + bn)
        nc.vector.tensor_reduce(
            out=bm32[:, sl], in_=src_ap[:, sl], axis=AX.X,
            op=ALU.max, apply_absolute_value=True,
        )
        # scale bits = max(exponent field, expfield(1e-23)) - (7 << 23)
        nc.vector.tensor_scalar(sb[:, sl], bm32[:, sl].bitcast(I32), EXPMASK,
                                None, ALU.bitwise_and)
        nc.vector.tensor_scalar(sb[:, sl], sb[:, sl], EGUARD, BIAS7,
                                ALU.max, ALU.subtract)
        # rscale bits = (254 << 23) - scale_bits  -> rscale = 2^(7-e)
        nc.vector.tensor_scalar(rb[:, sl], sb[:, sl], C254, -1,
                                ALU.subtract, ALU.mult)
        rsc = rb[:].bitcast(F32)[:, sl, None].to_broadcast((P, bn, 64))
        eng.tensor_tensor(tf3[:, sl], src_ap[:, sl], rsc, ALU.mult)
        nc.vector.tensor_scalar(t3[:, sl], tf3[:, sl], CROUND, CROUND,
                                ALU.add, ALU.subtract)
        nc.vector.tensor_scalar(t3[:, sl], t3[:, sl], 127.0, -127.0,
                                ALU.min, ALU.max)
        scb = sb[:].bitcast(F32)[:, sl, None].to_broadcast((P, bn, 64))
        seng.tensor_tensor(u3[:, sl], t3[:, sl], scb, ALU.mult)
        if piece_done is not None:
            piece_done(b0, bn)


def _interior(pad_tile):
    """[P, 56, 56] strided view (pitch 58) of the padded tile's interior."""
    base = 1 + PITCH + 1  # (h=0, w=0) -> index 1 + (h+1)*58 + (w+1)
    v = pad_tile[:, base : base + H * PITCH]
    return v.rearrange("p (h w) -> p h w", w=PITCH)[:, :, :W]


def _psv(ps):
    """[P, 8, 56] valid-column view of a [P, 464] PSUM chunk."""
    return ps[:].rearrange("p (r w) -> p r w", w=PITCH)[:, :, 1 : 1 + W]


def _emit_conv(nc, psum_pool, wk, src_pad, evict, preload=None):
    """3x3 conv via 9 accumulated matmuls per 8-row chunk; evict(c, psum).

    preload(c, ps): optionally pre-fill PSUM (residual); matmuls accumulate.
    """
    for c in range(NCHUNK):
        ps = psum_pool.tile([P, CHF], F32, tag="ps")
        if preload is not None:
            preload(c, ps)
        for k in range(9):
            kh, kw = divmod(k, 3)
            s = (c * 8 + kh) * PITCH + kw
            nc.tensor.matmul(
                ps[:], wk[k][:], src_pad[:, s : s + CHF],
                start=(k == 0 and preload is None), stop=(k == 8),
            )
        evict(c, ps)


def build_nc(nimg=NIMG):
    nc = bacc.Bacc("TRN2", target_bir_lowering=False, debug=False,
                   enable_asserts=False)

    x_d = nc.dram_tensor("x", [nimg, P, H, W], F32, kind="ExternalInput").ap()
    w1_d = nc.dram_tensor("w1", [P, P, 3, 3], F32, kind="ExternalInput").ap()
    w2_d = nc.dram_tensor("w2", [P, P, 3, 3], F32, kind="ExternalInput").ap()
    bn_d = {
        name: nc.dram_tensor(name, [P], F32, kind="ExternalInput").ap()
        for name in ("gamma1", "beta1", "mean1", "var1",
                     "gamma2", "beta2", "mean2", "var2")
    }
    out_d = nc.dram_tensor("out", [nimg, P, H, W], F32, kind="ExternalOutput").ap()

    with tile.TileContext(nc) as tc, ExitStack() as ctx:
        const = ctx.enter_context(tc.tile_pool(name="const", bufs=1))
        small = ctx.enter_context(tc.tile_pool(name="small", bufs=4))
        t16_p = ctx.enter_context(tc.tile_pool(name="t16", bufs=2))
        pads = ctx.enter_context(tc.tile_pool(name="pads", bufs=1))
        xraw_p = ctx.enter_context(tc.tile_pool(name="xraw", bufs=3))

        xraws = [None] * nimg

        def load(n):
            xraw = xraw_p.tile([P, HWF], F32, tag="xraw", name=f"xraw{n}")
            xraws[n] = xraw
            nc.sync.dma_start(xraw[:], x_d[n].rearrange("c h w -> c (h w)"))

        # first images stream in while weights are being prepared
        load(0)
        load(1)

        # ---- setup: BN constants, weights quant (+bn2 fold) + transpose ----
        with tc.tile_pool(name="setup", bufs=1) as setup, \
             tc.tile_pool(name="psum_setup", bufs=2, space="PSUM") as psum_setup:
            ident = const.tile([P, P], BF16, tag="ident")
            make_identity(nc, ident[:])
            eps_b = const.tile([P, 1], F32, tag="eps_b")
            nc.vector.memset(eps_b[:], BN_EPS)

            bnc = {}
            for name in ("gamma1", "beta1", "mean1", "var1",
                         "gamma2", "beta2", "mean2", "var2"):
                t = setup.tile([P, 1], F32, tag=f"bn_{name}")
                nc.sync.dma_start(t[:], bn_d[name][:, None])
                bnc[name] = t
            invb = []
            for i in ("1", "2"):
                s = setup.tile([P, 1], F32, tag=f"sd{i}")
                nc.scalar.activation(s[:], bnc[f"var{i}"][:], ACTF.Sqrt, bias=eps_b[:])
                r = setup.tile([P, 1], F32, tag=f"rs{i}")
                nc.vector.reciprocal(r[:], s[:])
                inv = const.tile([P, 1], F32, tag=f"inv{i}")
                nc.vector.tensor_tensor(inv[:], bnc[f"gamma{i}"][:], r[:], ALU.mult)
                mi = setup.tile([P, 1], F32, tag=f"mi{i}")
                nc.vector.tensor_tensor(mi[:], bnc[f"mean{i}"][:], inv[:], ALU.mult)
                b = const.tile([P, 1], F32, tag=f"b{i}")
                nc.vector.tensor_tensor(b[:], bnc[f"beta{i}"][:], mi[:], ALU.subtract)
                invb.append((inv, b))
            (inv1, b1), (inv2, b2) = invb

            wks = []
            for wi, w_d in enumerate((w1_d, w2_d)):
                wraw = setup.tile([P, WLEN], F32, tag=f"wraw{wi}")
                nc.sync.dma_start(wraw[:], w_d.rearrange("o i kh kw -> o (i kh kw)"))
                wq = setup.tile([P, WLEN], BF16, tag=f"wq{wi}")
                _emit_quant(nc, small, setup,
                            wraw[:].rearrange("p (b e) -> p b e", e=64),
                            wq[:].rearrange("p (b e) -> p b e", e=64),
                            NBW, [(0, NBW)], lane="mixed")
                if wi == 1:
                    # fold bn2's per-output-channel scale into w2
                    nc.scalar.activation(wq[:], wq[:], ACTF.Identity,
                                         scale=inv2[:])
                # per-offset lhsT tiles: w[k][i, o] = wq[o, i*9+k]
                wq_v = wq[:].rearrange("p (i k) -> p k i", k=9)
                wk = []
                for k in range(9):
                    pt = psum_setup.tile([P, P], BF16, tag="tps")
                    nc.tensor.transpose(pt[:], wq_v[:, k, :], ident[:])
                    wt = const.tile([P, P], BF16, tag=f"w{wi}k{k}")
                    nc.scalar.copy(wt[:], pt[:])
                    wk.append(wt)
                wks.append(wk)
            w1k, w2k = wks

        xq_pads = [pads.tile([P, PADLEN], BF16, tag=f"xqp{i}", name=f"xqp{i}")
                   for i in range(2)]
        mq_pads = [pads.tile([P, PADLEN], BF16, tag=f"mqp{i}", name=f"mqp{i}")
                   for i in range(NPAD)]
        # bf16 copies of x in the padded layout: residual preload source for
        # the identity matmul that seeds conv2's PSUM
        xr_pads = [pads.tile([P, PADLEN], BF16, tag=f"xrp{i}", name=f"xrp{i}")
                   for i in range(2)]
        for t in (*xq_pads, *mq_pads, *xr_pads):
            nc.scalar.memzero(t[:])

        u_p = ctx.enter_context(tc.tile_pool(name="u", bufs=2))
        mid_p = ctx.enter_context(tc.tile_pool(name="mid", bufs=2))
        out_p = ctx.enter_context(tc.tile_pool(name="outp", bufs=2))
        psum1_p = ctx.enter_context(tc.tile_pool(name="psum1", bufs=4, space="PSUM"))
        psum2_p = ctx.enter_context(tc.tile_pool(name="psum2", bufs=4, space="PSUM"))

        mids = [None] * nimg
        outs = [None] * nimg

        def quant1(n, lane="mixed"):
            u = u_p.tile([P, HWF], BF16, tag="u", name=f"u{n}")
            _emit_quant(nc, small, t16_p,
                        xraws[n][:].rearrange("p (b e) -> p b e", e=64),
                        u[:].rearrange("p (b e) -> p b e", e=64),
                        NBX, [(0, NBX)], lane=lane)
            nc.sync.dma_start(_interior(xq_pads[n % 2]),
                              u[:].rearrange("p (h w) -> p h w", w=W))

        def conv1(n):
            mid = mid_p.tile([P, HWF], F32, tag="mid", name=f"mid{n}")
            mids[n] = mid

            def evict1(c, ps):
                ov = mid[:, c * CHW : (c + 1) * CHW].rearrange(
                    "p (r w) -> p r w", w=W)
                nc.scalar.activation(ov, _psv(ps),
                                     ACTF.Relu, bias=b1[:], scale=inv1[:])

            _emit_conv(nc, psum1_p, w1k, xq_pads[n % 2][:], evict1)

        def quant2(n):
            # stage the bf16 residual into its padded tile (one ACT op; also
            # the f32->bf16 cast). Needed by conv2(n)'s preload matmuls.
            nc.scalar.copy(_interior(xr_pads[n % 2]),
                           xraws[n][:].rearrange("p (h w) -> p h w", w=W))
            u2 = u_p.tile([P, HWF], BF16, tag="u2", name=f"u2_{n}")
            # pieces aligned to evict1's chunks (4/7)
            iv = _interior(mq_pads[n % NPAD])
            u2v = u2[:].rearrange("p (h w) -> p h w", w=W)

            def reloc(b0, bn):
                r0, r1 = b0 * 64 // W, (b0 + bn) * 64 // W
                nc.sync.dma_start(iv[:, r0:r1, :], u2v[:, r0:r1, :])

            _emit_quant(nc, small, t16_p,
                        mids[n][:].rearrange("p (b e) -> p b e", e=64),
                        u2[:].rearrange("p (b e) -> p b e", e=64),
                        NBX, [(0, 28), (28, 21)], lane="dve_gs",
                        piece_done=reloc)

        def conv2(n):
            out_t = out_p.tile([P, HWF], F32, tag="out", name=f"out{n}")
            outs[n] = out_t
            xr_pad = xr_pads[n % 2]

            def preload(c, ps):
                # seed PSUM with the residual x via an identity matmul: runs
                # on the PE queue, so ordering with the accumulating conv
                # matmuls is guaranteed (no cross-engine PSUM write race)
                s = (c * 8 + 1) * PITCH + 1
                nc.tensor.matmul(ps[:], ident[:], xr_pad[:, s : s + CHF],
                                 start=True, stop=False)

            def evict2(c, ps):
                sl = slice(c * CHW, (c + 1) * CHW)
                ov = out_t[:, sl].rearrange("p (r w) -> p r w", w=W)
                # out = relu(bn2(conv2) + x): inv2 folded into w2, x in psum
                nc.scalar.activation(ov, _psv(ps), ACTF.Relu, bias=b2[:])

            _emit_conv(nc, psum2_p, w2k, mq_pads[n % NPAD][:], evict2,
                       preload=preload)
            for sl in (slice(0, 4 * CHW), slice(4 * CHW, HWF)):
                nc.sync.dma_start(
                    out_d[n].rearrange("c h w -> c (h w)")[:, sl],
                    out_t[:, sl])

        # Software-pipelined emission ordered by criticality: the
        # latency-critical quant2(n) -> conv2(n) chain first each iteration
        # (pure DVE lane), conv1(n+1) as PE filler, input load/quant two
        # images ahead (pure GpSimd lane after the DVE reduce head, so the
        # two in-order queues never block each other).
        quant1(0, lane="dve")
        quant1(1, lane="dve")
        conv1(0)
        for n in range(nimg):
            quant2(n)
            if n + 1 < nimg:
                conv1(n + 1)
            conv2(n)
            if n + 2 < nimg:
                load(n + 2)
                quant1(n + 2, lane="mixed")

    nc.compile()
    return nc


@lru_cache(maxsize=1)
def _get_nc():
    return build_nc(NIMG)


def kernel(x, w1, w2, gamma1, beta1, mean1, var1,
           gamma2, beta2, mean2, var2, _trace=False):
    f = lambda a: np.ascontiguousarray(np.asarray(a, dtype=np.float32))
    x = f(x)
    n_total = x.shape[0]
    assert n_total == N_CORES * NIMG, x.shape
    xs = x.reshape(N_CORES, NIMG, P, H, W)
    rep = {
        "w1": f(w1), "w2": f(w2),
        "gamma1": f(gamma1), "beta1": f(beta1), "mean1": f(mean1), "var1": f(var1),
        "gamma2": f(gamma2), "beta2": f(beta2), "mean2": f(mean2), "var2": f(var2),
    }
    in_maps = [{"x": np.ascontiguousarray(xs[c]), **rep} for c in range(N_CORES)]
    nc = _get_nc()
    res = run_bass_kernel_spmd(nc, in_maps, core_ids=list(range(N_CORES)),
                               trace=_trace)
    out = np.concatenate([res.results[c]["out"] for c in range(N_CORES)], axis=0)
    if _trace:
        kernel.last_result = res
    return out.reshape(n_total, P, H, W)
